# revision 26
# baseline (speedup 1.0000x reference)
"""Trainium2 Bass kernel: batch row-sharded grouped GEMM (MoE routing).

Contract: kernel(x, weight, num_inputs_per_group) takes FULL inputs
  x (32768, 2048) f32, weight (16, 2048, 2048) f32, num_inputs_per_group (16,) i32
and returns the FULL output (32768, 2048) f32, where token row i is multiplied
by weight[seg[i]] with seg = repeat(arange(16), num, total_repeat_length=32768)
(contiguous groups).

Distribution strategy (no collectives needed): tokens are split into contiguous
128-row blocks; each of the 8 cores gets an equal number of blocks plus the
weights for the experts its blocks use (expert/token parallelism — sanctioned
by the sharding hint since E=16 >= 8). Each core computes a dense grouped GEMM
locally and the host concatenates the per-core outputs.

Device kernel: fp32r matmuls (TF32-like input rounding, fp32 accumulation,
full PE rate). Host pre-lays-out both operands so every DMA moves multi-KB
contiguous chunks per partition:
  x  -> per-group tiles [128(d%128), 16(d//128), 256(token)]  (16 KB chunks)
  w  -> per-(slot, n-set) tiles [128, 16, 512]                (32 KB chunks)
Weight n-sets give n-granular dependencies: the first 4 MiB set unlocks every
token block's n=0 PSUM group ~12us after launch instead of the full 16 MiB
expert load gating the first block.
"""

import sys

sys.path.insert(0, "/opt/trn_rl_repo")

import numpy as np

import concourse.bacc as bacc
import concourse.mybir as mybir
from concourse.bass_utils import run_bass_kernel_spmd
from concourse.tile import TileContext
from concourse.tile_rust import add_dep_helper

N_TOK, D_IN, D_OUT, N_EXP = 32768, 2048, 2048, 16
NCORES = 8
PB = 128  # token block = PSUM partition count
NT = 512  # matmul moving free dim = one fp32 PSUM bank
KT = D_IN // PB  # 16 k-tiles
NTILES = D_OUT // NT  # 4 output column sets
MG_BLOCKS = 2  # token blocks per x group tile
MGT = MG_BLOCKS * PB  # tokens per group tile

# Fast path (bf16, weights resident): blocks covered by the n-major ramp
# prologue at the start, and the n-major epilogue block at the end.
RAMP_BLOCKS = 4

# Introspection hooks for test.py (harness just calls kernel()).
TRACE = False
LAST_RESULTS = None


def _seg_from_groups(num):
    """Replicate jnp.repeat(arange(E), num, total_repeat_length=N) semantics."""
    num = np.asarray(num, dtype=np.int64)
    reps = np.repeat(np.arange(N_EXP, dtype=np.int32), np.maximum(num, 0))
    if len(reps) >= N_TOK:
        return reps[:N_TOK]
    pad = reps[-1] if len(reps) else np.int32(0)
    return np.concatenate([reps, np.full(N_TOK - len(reps), pad, np.int32)])


def _run_groups(runs):
    """Split each run's blocks into m-groups of up to MG_BLOCKS blocks."""
    groups = []  # (run_idx, g_blocks)
    for ri, (_, nb) in enumerate(runs):
        b = 0
        while b < nb:
            g = min(MG_BLOCKS, nb - b)
            groups.append((ri, g))
            b += g
    return groups


def _build_nc(n_blocks_core, runs, n_slots):
    """Build the per-core SPMD kernel.

    runs: list of (slot, n_blocks) with sum(n_blocks) == n_blocks_core.
    Every core runs this same program; per-core data (x slice, slot->expert
    weight choice) lives in the input maps.
    """
    T_core = n_blocks_core * PB
    f32 = mybir.dt.float32
    f32r = mybir.dt.float32r
    bf16 = mybir.dt.bfloat16
    groups = _run_groups(runs)

    nc = bacc.Bacc("TRN2", target_bir_lowering=False, debug=False, num_devices=NCORES)
    xh = nc.dram_tensor("xh", [len(groups), PB, KT, MGT], f32r, kind="ExternalInput")
    w = nc.dram_tensor("w", [n_slots, NTILES, PB, KT, NT], f32r, kind="ExternalInput")
    out = nc.dram_tensor("out", [T_core, D_OUT], bf16, kind="ExternalOutput")

    with TileContext(nc) as tc:
        with (
            tc.tile_pool(name="wpool", bufs=4) as wpool,
            tc.tile_pool(name="xpool", bufs=4) as xpool,
            tc.tile_pool(name="opool", bufs=4) as opool,
            tc.tile_pool(name="pspool", bufs=7, space="PSUM") as pspool,
            tc.tile_pool(name="warmpool", bufs=1, space="PSUM") as warmpool,
        ):
            # Warm-up: ~30 throwaway matmuls keep the PE busy (and its HAM
            # clock gate at 8/8) through the ~20us HBM-bound ramp while the
            # first weight set and x tiles stream in. fp32 (not fp32r) so the
            # memset-produced scratch needs no fp32r rounding producer.
            wsrc = xpool.tile([PB, PB], f32, name="warm_src", tag="warm")
            nc.vector.memset(wsrc, 0.0)
            wps = warmpool.tile([PB, PB], f32, name="warm_ps", tag="warm_ps")
            for _ in range(30):
                nc.tensor.matmul(wps, wsrc, wsrc, start=True, stop=True)
            run_group0 = []
            g0 = 0
            for ri in range(len(runs)):
                run_group0.append(g0)
                g0 += sum(1 for r, _ in groups if r == ri)

            blk = 0
            for ri, (slot, nb) in enumerate(runs):
                # Two passes over this run's tokens: pass 0 consumes weight
                # n-sets {0,1}, pass 1 consumes {2,3}. The PE executes in
                # program order, so each 4 MiB n-set only gates work that
                # genuinely needs it, and each set has a half-run (~100us) of
                # compute as prefetch slack — x is re-streamed per pass to
                # keep SBUF small (DMA has the headroom; PE is the
                # bottleneck). Weight n-sets go on the SP HWDGE ring; x/out
                # use the ACT ring so they never queue behind a weight
                # stream. Each n-set is split into two k-half DMAs so the
                # first PSUM group can start after ~2 MiB. The s2/s3 DMAs are
                # artificially made dependent on early pass-0 matmuls: HBM
                # bandwidth is the ramp bottleneck, and without the dep they
                # stream immediately and starve the x/s0/s1 loads the ramp
                # actually needs.
                wt = []
                w_dmas = []
                for n in range(NTILES):
                    t = wpool.tile(
                        [PB, KT, NT], f32r, name=f"w_s{slot}_n{n}", tag="w"
                    )
                    # First run's n0 in k-quarters (finer dependency grain
                    # during the supply-limited ramp); everything else in
                    # k-halves.
                    pieces = 4 if (ri == 0 and n == 0) else 2
                    kp = KT // pieces
                    ds = []
                    for q in range(pieces):
                        ds.append(
                            nc.sync.dma_start(
                                out=t[:, q * kp : (q + 1) * kp, :],
                                in_=w[slot, n, :, q * kp : (q + 1) * kp, :],
                            )
                        )
                    wt.append(t)
                    w_dmas.append(tuple(ds))
                pass_sets = [[0, 1], [2, 3]]
                chunk_first_mm = {}  # chunk ordinal in pass 0 -> first MM inst
                for p, nset in enumerate(pass_sets):
                    gi = run_group0[ri]
                    chunk = 0
                    b = 0
                    nchunks_run = sum(1 for r, _ in groups if r == ri)
                    # n-sequential ramp prologue: the first PRO chunks run all
                    # their first-nset psum groups back-to-back, then their
                    # second-nset groups — so weight n-sets are consumed in
                    # DMA arrival order while HBM supply is still the
                    # bottleneck (~430 GB/s from ~9 us; a 4 MiB n-set takes
                    # ~10 us to arrive but one psum group eats it in 3.4 us).
                    PRO = (
                        3
                        if (ri == 0 and p == 0 and len(nset) > 1 and nchunks_run >= 5)
                        else 0
                    )
                    deferred = []
                    while b < nb:
                        _, g = groups[gi]
                        xt = xpool.tile(
                            [PB, KT, MGT], f32r, name=f"xt_{gi}_{p}", tag="xt"
                        )
                        if ri == 0 and p == 0 and chunk == 0:
                            # First x tile in k-halves: the first matmul only
                            # gates on 1 MiB of x, not 2.
                            xkh = KT // 2
                            nc.scalar.dma_start(
                                out=xt[:, :xkh, :], in_=xh[gi][:, :xkh, :]
                            )
                            nc.scalar.dma_start(
                                out=xt[:, xkh:, :], in_=xh[gi][:, xkh:, :]
                            )
                        else:
                            nc.scalar.dma_start(out=xt, in_=xh[gi])

                        # (n, mb, k) order: each PSUM group's 16 matmuls
                        # accumulate over k, and the second weight n-set of a
                        # pass isn't touched until ~10us of compute into it.
                        def emit_group_store(xt_, g_, base_b, j):
                            """nset j's psum groups for one chunk, with
                            per-block copy + per-nset store."""
                            n = nset[j]
                            first = None
                            for mb in range(g_):
                                ps = pspool.tile([PB, NT], f32, name="ps", tag="ps")
                                for k in range(KT):
                                    mm = nc.tensor.matmul(
                                        ps,
                                        xt_[:, k, mb * PB : (mb + 1) * PB],
                                        wt[n][:, k, :],
                                        start=(k == 0),
                                        stop=(k == KT - 1),
                                    )
                                    if first is None:
                                        first = mm
                                ot = opool.tile(
                                    [PB, NT],
                                    bf16,
                                    name=f"o_{blk + base_b + mb}_{n}",
                                    tag="o",
                                )
                                nc.vector.tensor_copy(out=ot, in_=ps)
                                row = (blk + base_b + mb) * PB
                                nc.scalar.dma_start(
                                    out=out[row : row + PB, n * NT : (n + 1) * NT],
                                    in_=ot,
                                )
                            return first

                        all_js = list(range(len(nset)))
                        if chunk < PRO:
                            mm0 = emit_group_store(xt, g, b, all_js[0])
                            chunk_first_mm[chunk] = mm0
                            deferred.append((xt, g, b))
                        else:
                            if deferred:
                                for xt_, g_, b_ in deferred:
                                    for j in all_js[1:]:
                                        emit_group_store(xt_, g_, b_, j)
                                deferred = []
                            for j in all_js:
                                mm0 = emit_group_store(xt, g, b, j)
                                if p == 0 and j == all_js[0]:
                                    chunk_first_mm[chunk] = mm0
                        gi += 1
                        chunk += 1
                        b += g
                    for xt_, g_, b_ in deferred:
                        for j in all_js[1:]:
                            emit_group_store(xt_, g_, b_, j)
                    deferred = []
                # Hold back this run's s2/s3 streams until its pass-0 compute
                # is underway (see comment above).
                nchunks = chunk
                for n, anchor in ((2, 1), (3, 2)):
                    a = chunk_first_mm.get(min(anchor, nchunks - 1))
                    if a is not None and nchunks > 2:
                        for dd in w_dmas[n]:
                            add_dep_helper(
                                dd.ins,
                                a.ins,
                                sync=True,
                                reason="stagger weight n-set stream behind ramp",
                            )
                blk += nb
    nc.compile()
    return nc


def _build_nc_fast(n_blocks_core):
    """bf16 fast path: both experts' weights fully SBUF-resident.

    Requires runs == [(0, nb/2), (1, nb/2)] (two equal expert runs per core,
    the aligned equal-groups case). Layout per core:
      xh  [G, 128(d%128), 16(d//128), 256(token)]   bf16 (G = nb/2 groups)
      w   [2, 4(nset), 128, 16, 512]                bf16 (16 KB/partition/tile)
      out [T_core, 2048]                            bf16 (host upcasts)

    Loop structure:
      - 30 fp32 warm-up matmuls spin the PE/HAM clock up while the first
        weight k-halves and x tiles stream in.
      - Blocks 0..RAMP_BLOCKS-1 and the last block: n-set-major psum groups
        (k inner) so weight n-sets are consumed in DMA arrival order during
        the HBM-bound ramp, and the tail block finishes n3 last with n0-n2
        already copied + stored.
      - All other blocks: k-outer / n-inner, so the PE's stationary operand
        (the x block) is identical for 4 consecutive matmuls; 4 psum banks
        accumulate in parallel and the 8-bank pool double-buffers across
        blocks with no boundary stall.
    All four weight n-set tiles of a block's expert must be resident before
    its k-major loop starts; the n-major ramp covers exactly the window
    where that isn't true yet.
    """
    assert n_blocks_core % (2 * MG_BLOCKS) == 0
    T_core = n_blocks_core * PB
    G = n_blocks_core // MG_BLOCKS
    nb_slot = n_blocks_core // 2
    f32 = mybir.dt.float32
    bf16 = mybir.dt.bfloat16

    nc = bacc.Bacc("TRN2", target_bir_lowering=False, debug=False, num_devices=NCORES)
    xh = nc.dram_tensor("xh", [G, PB, KT, MGT], bf16, kind="ExternalInput")
    w = nc.dram_tensor("w", [2, NTILES, PB, KT, NT], bf16, kind="ExternalInput")
    out = nc.dram_tensor("out", [T_core, D_OUT], bf16, kind="ExternalOutput")

    with TileContext(nc) as tc:
        with (
            tc.tile_pool(name="wpool", bufs=2 * NTILES) as wpool,
            tc.tile_pool(name="xpool", bufs=3) as xpool,
            tc.tile_pool(name="opool", bufs=3) as opool,
            tc.tile_pool(name="pspool", bufs=8, space="PSUM") as pspool,
        ):
            # Warm-up matmuls: keep the PE busy (HAM gate at 8/8, full
            # p-state) through the HBM-bound ramp. fp32 scratch, discarded.
            wsrc = xpool.tile([PB, PB], f32, name="warm_src", tag="warm")
            nc.vector.memset(wsrc, 0.0)
            wps = pspool.tile([PB, NT], f32, name="ps", tag="ps")
            for _ in range(20):
                nc.tensor.matmul(wps[:, :PB], wsrc, wsrc, start=True, stop=True)

            # Resident weights: one full-tile DMA per (slot, n-set), even
            # n-sets on the SP HWDGE ring, odd on the GpSimd SWDGE ring.
            # Big transfers amortize per-DMA ring overhead (1 MiB pieces
            # measured only ~160 GB/s aggregate; the per-core cap is
            # ~435 GB/s), and the two rings stream n0 and n1 in parallel so
            # both are resident by the time warm-up ends. Ring order alone
            # keeps slot 1 behind slot 0.
            wt = []
            for s in range(2):
                row = []
                for n in range(NTILES):
                    t = wpool.tile([PB, KT, NT], bf16, name=f"w_s{s}_n{n}", tag="w")
                    ring = nc.sync if n % 2 == 0 else nc.gpsimd
                    ring.dma_start(out=t, in_=w[s, n])
                    row.append(t)
                wt.append(row)

            xt_of_group = {}

            def get_xt(gi):
                if gi not in xt_of_group:
                    t = xpool.tile([PB, KT, MGT], bf16, name=f"xt_{gi}", tag="xt")
                    nc.scalar.dma_start(out=t, in_=xh[gi])
                    xt_of_group[gi] = t
                return xt_of_group[gi]

            def stationary(b):
                gi, mb = b // MG_BLOCKS, b % MG_BLOCKS
                return get_xt(gi)[:, :, mb * PB : (mb + 1) * PB]

            def slot_of(b):
                return b // nb_slot

            def emit_group(b, n):
                """One n-major psum group: 16 k-matmuls + copy + store."""
                xs = stationary(b)
                ps = pspool.tile([PB, NT], f32, name="ps", tag="ps")
                for k in range(KT):
                    nc.tensor.matmul(
                        ps,
                        xs[:, k, :],
                        wt[slot_of(b)][n][:, k, :],
                        start=(k == 0),
                        stop=(k == KT - 1),
                    )
                ot = opool.tile([PB, NT], bf16, name=f"o_{b}_{n}", tag="o")
                nc.vector.tensor_copy(out=ot, in_=ps)
                nc.scalar.dma_start(
                    out=out[b * PB : (b + 1) * PB, n * NT : (n + 1) * NT], in_=ot
                )

            # --- ramp prologue: n-set-major over the first RAMP_BLOCKS ----
            for n in range(NTILES):
                for b in range(RAMP_BLOCKS):
                    emit_group(b, n)

            # --- steady state: k-outer / n-inner per block -----------------
            for b in range(RAMP_BLOCKS, n_blocks_core - 1):
                xs = stationary(b)
                s = slot_of(b)
                pss = [
                    pspool.tile([PB, NT], f32, name="ps", tag="ps")
                    for _ in range(NTILES)
                ]
                for k in range(KT):
                    for n in range(NTILES):
                        nc.tensor.matmul(
                            pss[n],
                            xs[:, k, :],
                            wt[s][n][:, k, :],
                            start=(k == 0),
                            stop=(k == KT - 1),
                        )
                ot = opool.tile([PB, D_OUT], bf16, name=f"o_{b}", tag="o")
                for n in range(NTILES):
                    nc.vector.tensor_copy(out=ot[:, n * NT : (n + 1) * NT], in_=pss[n])
                nc.scalar.dma_start(out=out[b * PB : (b + 1) * PB, :], in_=ot)

            # --- tail epilogue: n-major so the kernel ends on a 256 KB store
            for n in range(NTILES):
                emit_group(n_blocks_core - 1, n)

    nc.compile()
    return nc


def _host_layout_x_fast(x_core):
    """Pack a core's tokens [T, 2048] f32 into bf16 tiles [G, 128, 16, 256]."""
    import ml_dtypes

    T = x_core.shape[0]
    G = T // MGT
    # (g, t, k, p) -> (g, p, k, t)
    xh = (
        x_core.reshape(G, MGT, KT, PB)
        .transpose(0, 3, 2, 1)
        .astype(ml_dtypes.bfloat16)
    )
    return np.ascontiguousarray(xh)


def _host_layout_w_fast(w_slots):
    """Pack slot weights [2, D, O] f32 into bf16 tiles [2, 4, 128, 16, 512]."""
    import ml_dtypes

    S = w_slots.shape[0]
    # (s, k, p, n, j) -> (s, n, p, k, j)
    return np.ascontiguousarray(
        w_slots.reshape(S, KT, PB, NTILES, NT)
        .transpose(0, 3, 2, 1, 4)
        .astype(ml_dtypes.bfloat16)
    )


def _host_layout_x(x_core, runs):
    """Pack a core's tokens [T, D] into group tiles [NG, 128, 16, 256]."""
    groups = _run_groups(runs)
    xh = np.zeros((len(groups), PB, KT, MGT), dtype=np.float32)
    t0 = 0
    for i, (_, g) in enumerate(groups):
        gt = g * PB
        blockT = x_core[t0 : t0 + gt]  # [gt, D]
        # (t, k, p) -> (p, k, t)
        xh[i, :, :, :gt] = blockT.reshape(gt, KT, PB).transpose(2, 1, 0)
        t0 += gt
    return np.ascontiguousarray(xh)


def _host_layout_w(w_slots):
    """Pack slot weights [S, D, O] into n-set tiles [S, 4, 128, 16, 512]."""
    S = w_slots.shape[0]
    # (s, k, p, n, j) -> (s, n, p, k, j)
    return np.ascontiguousarray(
        w_slots.reshape(S, KT, PB, NTILES, NT).transpose(0, 3, 2, 1, 4)
    )


def kernel(x, weight, num_inputs_per_group):
    global LAST_RESULTS
    x = np.ascontiguousarray(np.asarray(x, dtype=np.float32))
    weight = np.ascontiguousarray(np.asarray(weight, dtype=np.float32))
    seg = _seg_from_groups(num_inputs_per_group)

    # --- plan: map 128-token blocks to experts ---------------------------------
    aligned = all(
        np.all(seg[i * PB : (i + 1) * PB] == seg[i * PB]) for i in range(N_TOK // PB)
    )
    if aligned:
        block_expert = seg[::PB].astype(np.int64)  # (256,)
        block_tokens = None  # identity: block b covers rows [b*128, (b+1)*128)
    else:
        # Generic fallback: pad each contiguous expert segment to a 128 multiple
        # via a host-side gather; output rows are scattered back afterwards.
        bounds = np.flatnonzero(np.diff(seg)) + 1
        starts = np.concatenate([[0], bounds])
        ends = np.concatenate([bounds, [N_TOK]])
        blocks, experts = [], []
        for s, e in zip(starts, ends):
            idx = np.arange(s, e, dtype=np.int64)
            padded = -np.ones(int(np.ceil(len(idx) / PB)) * PB, dtype=np.int64)
            padded[: len(idx)] = idx
            for b0 in range(0, len(padded), PB):
                blocks.append(padded[b0 : b0 + PB])
                experts.append(int(seg[s]))
        while len(blocks) % NCORES:
            blocks.append(-np.ones(PB, dtype=np.int64))
            experts.append(0)
        block_tokens = np.stack(blocks)  # (n_blocks, 128) token ids, -1 = pad
        block_expert = np.asarray(experts, dtype=np.int64)

    n_blocks = len(block_expert)
    n_blocks_core = n_blocks // NCORES
    per_core_experts = block_expert.reshape(NCORES, n_blocks_core)

    # Run-length encode each core's block->expert map; if all cores share the
    # same run-length pattern we can use compact per-run weight slots.
    def rle(v):
        runs = []
        for e in v:
            if runs and runs[-1][0] == e:
                runs[-1][1] += 1
            else:
                runs.append([int(e), 1])
        return runs

    core_runs = [rle(per_core_experts[c]) for c in range(NCORES)]
    lengths0 = [n for _, n in core_runs[0]]
    if all([n for _, n in core_runs[c]] == lengths0 for c in range(NCORES)):
        runs = [(s, n) for s, (_, n) in enumerate(core_runs[0])]
        slot_experts = [[e for e, _ in core_runs[c]] for c in range(NCORES)]
    else:
        runs = [(b, 1) for b in range(n_blocks_core)]
        slot_experts = [list(per_core_experts[c]) for c in range(NCORES)]
    n_slots = len(runs)

    # Fast path: aligned, two equal expert runs per core (the equal-group
    # routing case) -> bf16 kernel with both experts' weights SBUF-resident.
    # Cycle-normalized (the PE clock varies 2.0-2.4 GHz run to run), bf16
    # matmuls cost 518 PE cycles vs fp32r's 544, and bf16 halves HBM traffic.
    fast = (
        block_tokens is None
        and n_slots == 2
        and runs[0][1] == runs[1][1]
        and n_blocks_core % (2 * MG_BLOCKS) == 0
        and n_blocks_core >= 2 * RAMP_BLOCKS + 2
    )

    # --- per-core inputs -------------------------------------------------------
    in_maps = []
    for c in range(NCORES):
        if block_tokens is None:
            rows = slice(c * n_blocks_core * PB, (c + 1) * n_blocks_core * PB)
            xc = x[rows]
        else:
            tok = block_tokens[c * n_blocks_core : (c + 1) * n_blocks_core].ravel()
            xc = np.where(tok[:, None] >= 0, x[np.maximum(tok, 0)], 0.0).astype(
                np.float32
            )
        if fast:
            in_maps.append(
                {
                    "xh": _host_layout_x_fast(xc),
                    "w": _host_layout_w_fast(weight[slot_experts[c]]),
                }
            )
        else:
            in_maps.append(
                {
                    "xh": _host_layout_x(xc, runs),
                    "w": _host_layout_w(weight[slot_experts[c]]),
                }
            )

    if fast:
        nc = _build_nc_fast(n_blocks_core)
    else:
        nc = _build_nc(n_blocks_core, runs, n_slots)
    res = run_bass_kernel_spmd(nc, in_maps, core_ids=list(range(NCORES)), trace=TRACE)
    LAST_RESULTS = res

    # --- unshard ---------------------------------------------------------------
    outs = [np.asarray(res.results[c]["out"], dtype=np.float32) for c in range(NCORES)]
    if block_tokens is None:
        return np.concatenate(outs, axis=0)
    full = np.zeros((N_TOK, D_OUT), dtype=np.float32)
    flat_tok = block_tokens.ravel()
    flat_out = np.concatenate(outs, axis=0)
    valid = flat_tok >= 0
    full[flat_tok[valid]] = flat_out[valid]
    return full

